# revision 1
# baseline (speedup 1.0000x reference)
"""Cross-layer transcoder kernel for Trainium2 (8 NeuronCores, SPMD).

Math (from the reference):
    feats[l] = relu(x[l] @ W_enc[l].T + b_enc[l])          # [B, F] per layer
    recon[j] = sum_{i<=j} feats[i] @ W_dec[i, j] + b_dec[j] # [B, D] per layer

Sharding: the transcoder feature dim F=4096 is split across the 8 cores
(512 features each). Each core encodes its feature slice for all layers and
computes a partial reconstruction for every destination layer; the partials
are summed on the host (the gather/unshard step), where b_dec is also added.

Device layout notes:
  - All matmul operands are pre-transposed/packed on the host so every DMA is
    a contiguous [128, *] tile load and the PE contraction dim (d for encode,
    f for decode) lands on the partition axis.
  - Matmul inputs are cast to bf16 on the host; accumulation is fp32 in PSUM,
    the bias-add + relu runs in fp32 on the scalar engine. Measured end-to-end
    relative error vs the fp32 reference is ~3e-3.
  - Only the 36 upper-triangular (i, j) pairs of W_dec are ever transferred
    or computed.
"""

import os

import numpy as np
import ml_dtypes

L = 8          # n_layers
B = 1024       # n_pos
D = 768        # d_model
F = 4096       # d_transcoder
NCORES = 8
FL = F // NCORES   # features per core = 512
P = 128
KD = D // P        # 6  encode contraction chunks
MF = FL // P       # 4  feature chunks per core
MD = D // P        # 6  decode output chunks
NB = B // 512      # 2  position chunks of 512
PAIRS = [(i, j) for j in range(L) for i in range(j + 1)]
NPAIR = len(PAIRS)  # 36

BF16 = ml_dtypes.bfloat16

# Filled by the first kernel() call; reused afterwards.
_PROGRAM = None
# Stash of the most recent run's profiling results (test.py reads these).
LAST_EXEC_NS = None
LAST_RESULTS = None


def _build_program():
    import concourse.bacc as bacc
    import concourse.mybir as mybir
    import concourse.tile as tile

    nc = bacc.Bacc("TRN2", target_bir_lowering=False, debug=False)
    bf = mybir.dt.bfloat16
    f32 = mybir.dt.float32

    xT_d = nc.dram_tensor("xT", [L, KD, P, B], bf, kind="ExternalInput")
    wencT_d = nc.dram_tensor("wencT", [L, KD, P, FL], bf, kind="ExternalInput")
    benc_d = nc.dram_tensor("benc", [L, MF, P, 1], f32, kind="ExternalInput")
    wdec_d = nc.dram_tensor("wdec", [NPAIR, MF, P, D], bf, kind="ExternalInput")
    out_d = nc.dram_tensor("outT", [L, D, B], f32, kind="ExternalOutput")

    relu = mybir.ActivationFunctionType.Relu

    with tile.TileContext(nc) as tc:
        with (
            tc.tile_pool(name="feats", bufs=1) as feats_pool,
            tc.tile_pool(name="benc", bufs=1) as benc_pool,
            tc.tile_pool(name="xt", bufs=12) as xt_pool,
            tc.tile_pool(name="wenc", bufs=12) as wenc_pool,
            tc.tile_pool(name="wdec", bufs=44) as wdec_pool,
            tc.tile_pool(name="outs", bufs=8) as out_pool,
            tc.tile_pool(name="psum", bufs=8, space="PSUM") as psum_pool,
        ):
            # Warm up the tensor engine during the prologue DMA fill: the
            # first real matmul lands ~11us in, and HAM otherwise holds the
            # PE at 1.2 GHz for its first ~3.4us of work. These dummy
            # matmuls depend only on a memset tile, so they run from t~0.
            warm = feats_pool.tile([P, 512], bf, name="warm")
            nc.vector.memset(warm, 0)
            wps = psum_pool.tile([P, 512], f32, name="wps", tag="psum")
            for w in range(40):
                nc.tensor.matmul(
                    wps,
                    lhsT=warm[:, :P],
                    rhs=warm,
                    start=(w == 0),
                    stop=(w == 39),
                )

            feats = {}
            for j in range(L):
                # ---------- encode layer j into feats[(j, mf)] ----------
                xts = []
                wes = []
                for kd in range(KD):
                    xt = xt_pool.tile([P, B], bf, name="xt", tag="xt")
                    nc.sync.dma_start(xt, xT_d[j, kd])
                    xts.append(xt)
                    we = wenc_pool.tile([P, FL], bf, name="we", tag="we")
                    nc.sync.dma_start(we, wencT_d[j, kd])
                    wes.append(we)
                for mf in range(MF):
                    bt = benc_pool.tile([P, 1], f32, name=f"benc_{j}_{mf}")
                    nc.scalar.dma_start(bt, benc_d[j, mf])
                    ft = feats_pool.tile([P, B], bf, name=f"feat_{j}_{mf}")
                    feats[(j, mf)] = ft
                    for nb in range(NB):
                        ps = psum_pool.tile([P, 512], f32, name="ps", tag="psum")
                        for kd in range(KD):
                            nc.tensor.matmul(
                                ps,
                                lhsT=wes[kd][:, mf * P:(mf + 1) * P],
                                rhs=xts[kd][:, nb * 512:(nb + 1) * 512],
                                start=(kd == 0),
                                stop=(kd == KD - 1),
                            )
                        nc.scalar.activation(
                            ft[:, nb * 512:(nb + 1) * 512], ps, relu, bias=bt
                        )

                # ---------- decode destination layer j ----------
                nmm = (j + 1) * MF
                wts = {}
                for i in range(j + 1):
                    pidx = j * (j + 1) // 2 + i
                    for kf in range(MF):
                        wt = wdec_pool.tile([P, D], bf, name="wd", tag="wd")
                        nc.sync.dma_start(wt, wdec_d[pidx, kf])
                        wts[(i, kf)] = wt
                for md in range(MD):
                    for nb in range(NB):
                        ps = psum_pool.tile([P, 512], f32, name="ps", tag="psum")
                        c = 0
                        for i in range(j + 1):
                            for kf in range(MF):
                                nc.tensor.matmul(
                                    ps,
                                    lhsT=wts[(i, kf)][:, md * P:(md + 1) * P],
                                    rhs=feats[(i, kf)][:, nb * 512:(nb + 1) * 512],
                                    start=(c == 0),
                                    stop=(c == nmm - 1),
                                )
                                c += 1
                        ot = out_pool.tile([P, 512], f32, name="ot", tag="ot")
                        nc.vector.tensor_copy(ot, ps)
                        nc.scalar.dma_start(
                            out_d[j, md * P:(md + 1) * P, nb * 512:(nb + 1) * 512], ot
                        )

    nc.compile()
    return nc


def _prepare_inputs(x, W_enc, b_enc, W_dec):
    """Host-side shard + pack + cast. Returns in_maps for the 8 cores."""
    xT = np.ascontiguousarray(x.transpose(0, 2, 1)).astype(BF16).reshape(L, KD, P, B)
    in_maps = []
    for c in range(NCORES):
        s = slice(c * FL, (c + 1) * FL)
        wencT = (
            np.ascontiguousarray(W_enc[:, s, :].transpose(0, 2, 1))
            .astype(BF16)
            .reshape(L, KD, P, FL)
        )
        benc = np.ascontiguousarray(b_enc[:, s], dtype=np.float32).reshape(L, MF, P, 1)
        wdec = np.empty((NPAIR, MF, P, D), dtype=BF16)
        for pidx, (i, j) in enumerate(PAIRS):
            wdec[pidx] = W_dec[i, j, s, :].astype(BF16).reshape(MF, P, D)
        in_maps.append({"xT": xT, "wencT": wencT, "benc": benc, "wdec": wdec})
    return in_maps


def kernel(x, W_enc, b_enc, W_dec, b_dec):
    global _PROGRAM, LAST_EXEC_NS, LAST_RESULTS
    from concourse import bass_utils

    x = np.asarray(x)
    W_enc = np.asarray(W_enc)
    b_enc = np.asarray(b_enc)
    W_dec = np.asarray(W_dec)
    b_dec = np.asarray(b_dec)

    if _PROGRAM is None:
        _PROGRAM = _build_program()
    nc = _PROGRAM

    in_maps = _prepare_inputs(x, W_enc, b_enc, W_dec)

    trace = os.environ.get("KERNEL_TRACE", "0") == "1"
    res = bass_utils.run_bass_kernel_spmd(
        nc, in_maps, core_ids=list(range(NCORES)), trace=trace
    )
    LAST_EXEC_NS = res.exec_time_ns
    LAST_RESULTS = res

    acc = np.zeros((L, D, B), dtype=np.float32)
    for r in res.results:
        acc += np.asarray(r["outT"], dtype=np.float32)
    out = acc.transpose(0, 2, 1) + b_dec.astype(np.float32)[:, None, :]
    return np.ascontiguousarray(out, dtype=np.float32)



# revision 3
# speedup vs baseline: 1.0538x; 1.0538x over previous
"""Cross-layer transcoder kernel for Trainium2 (8 NeuronCores, SPMD).

Math (from the reference):
    feats[l] = relu(x[l] @ W_enc[l].T + b_enc[l])           # [B, F] per layer
    recon[j] = sum_{i<=j} feats[i] @ W_dec[i, j] + b_dec[j]  # [B, D] per layer

Sharding: the transcoder feature dim F=4096 is split across the 8 cores
(512 features each). Each core encodes its feature slice for all layers and
computes a partial reconstruction for every destination layer; the partials
are summed on the host (the gather/unshard step), where b_dec is also added.

Schedule (v2): all 8 layer encodes run first, then the 8 triangular decodes.
This front-loads the cheap-to-feed compute (encode needs 2.4MB/layer) and
gives the W_dec stream (28MB/core) an 80us runway to prefetch, so the PE
never waits on HBM after the first ~15us.

DMA layout (v2): every operand is host-packed so one dma_start moves one
whole logical tile with 6-12KB contiguous lines:
  - x:      1 load per layer   [128, 6144] bf16 (12KB lines)
  - W_enc:  1 load per layer   [128, 3072] bf16 (6KB lines)
  - b_enc:  1 load total       [128, 32]   f32
  - W_dec:  1 load per (i,j)   [128, 3072] bf16 (6KB lines), 36 pairs
  - out:    2 stores per (j,nb) of a [128, 3072] bf16 staging tile
This cuts dma_start count 400 -> ~85 (each costs ~650ns of serial issue
time on the Sync/Scalar engine) and roughly doubles effective early HBM
bandwidth vs 1-2KB lines.

Numerics: matmuls in bf16, fp32 PSUM accumulation, bias+relu in fp32 on
the scalar engine writing bf16 feats. Output partials stored as bf16
(error contribution ~0.2% vs the 2e-2 gate; measured total ~3e-3).
"""

import os

import numpy as np
import ml_dtypes

L = 8          # n_layers
B = 1024       # n_pos
D = 768        # d_model
F = 4096       # d_transcoder
NCORES = 8
FL = F // NCORES   # features per core = 512
P = 128
KD = D // P        # 6  encode contraction chunks
MF = FL // P       # 4  feature chunks per core
MD = D // P        # 6  decode output chunks
NB = B // 512      # 2  position chunks of 512
PAIRS = [(i, j) for j in range(L) for i in range(j + 1)]
NPAIR = len(PAIRS)  # 36

WDEC_BUFS = 14      # W_dec SBUF ring slots (6KB/partition each)
N_WARMUP = 16       # PE warmup matmuls bridging the DMA prologue

# How many W_dec pair-loads to enqueue behind each encode layer's loads on
# the sync DMA queue. Cumulative total must stay <= WDEC_BUFS or the queue
# would block on a ring slot that is only freed by decode (which runs after
# all encodes -> deadlock).
_PREFETCH_QUOTA = [1, 2, 3, 4, 4, 0, 0, 0]
assert sum(_PREFETCH_QUOTA) <= WDEC_BUFS

BF16 = ml_dtypes.bfloat16

# Filled by the first kernel() call; reused afterwards.
_PROGRAM = None
# Stash of the most recent run's profiling results (test.py reads these).
LAST_EXEC_NS = None
LAST_RESULTS = None


def _build_program():
    import concourse.bacc as bacc
    import concourse.mybir as mybir
    import concourse.tile as tile

    nc = bacc.Bacc("TRN2", target_bir_lowering=False, debug=False)
    bf = mybir.dt.bfloat16
    f32 = mybir.dt.float32

    xt_d = nc.dram_tensor("xt", [L, P, KD * B], bf, kind="ExternalInput")
    wenc_d = nc.dram_tensor("wenc", [L, P, KD * FL], bf, kind="ExternalInput")
    benc_d = nc.dram_tensor("benc", [P, L * MF], f32, kind="ExternalInput")
    wdec_d = nc.dram_tensor("wdec", [NPAIR, P, MF * D], bf, kind="ExternalInput")
    out_d = nc.dram_tensor("outT", [L, NB, P, MD * 512], bf, kind="ExternalOutput")

    relu = mybir.ActivationFunctionType.Relu

    with tile.TileContext(nc) as tc:
        with (
            tc.tile_pool(name="feats", bufs=1) as feats_pool,
            tc.tile_pool(name="benc", bufs=1) as benc_pool,
            tc.tile_pool(name="xt", bufs=2) as xt_pool,
            tc.tile_pool(name="wenc", bufs=2) as wenc_pool,
            tc.tile_pool(name="wdec", bufs=WDEC_BUFS) as wdec_pool,
            tc.tile_pool(name="stage", bufs=2) as stage_pool,
            tc.tile_pool(name="psum", bufs=8, space="PSUM") as psum_pool,
        ):
            # b_enc: one tiny load, column (j*MF+mf) is the bias vector for
            # that feature chunk.
            bt = benc_pool.tile([P, L * MF], f32, name="benc_t")
            nc.scalar.dma_start(bt, benc_d[:, :])

            # PE warmup during the DMA prologue: first real matmul data
            # lands ~14us in; these dummies keep HAM from holding the PE
            # at 1.2 GHz when the real stream starts.
            warm = benc_pool.tile([P, 512], bf, name="warm")
            nc.vector.memset(warm, 0)
            wps = psum_pool.tile([P, 512], f32, name="wps", tag="psum")
            for w in range(N_WARMUP):
                nc.tensor.matmul(
                    wps,
                    lhsT=warm[:, :P],
                    rhs=warm,
                    start=(w == 0),
                    stop=(w == N_WARMUP - 1),
                )

            feats = {}
            wd_tiles = {}
            emitted = 0

            def emit_wdec(pidx):
                wt = wdec_pool.tile([P, MF * D], bf, name="wd", tag="wd")
                nc.sync.dma_start(wt, wdec_d[pidx])
                wd_tiles[pidx] = wt

            # ---------------- encode all layers ----------------
            for j in range(L):
                xt = xt_pool.tile([P, KD * B], bf, name="xt", tag="xt")
                nc.sync.dma_start(xt, xt_d[j])
                we = wenc_pool.tile([P, KD * FL], bf, name="we", tag="we")
                nc.sync.dma_start(we, wenc_d[j])
                for _ in range(_PREFETCH_QUOTA[j]):
                    emit_wdec(emitted)
                    emitted += 1

                for mf in range(MF):
                    ft = feats_pool.tile([P, B], bf, name=f"feat_{j}_{mf}")
                    feats[(j, mf)] = ft
                    bcol = j * MF + mf
                    for nb in range(NB):
                        ps = psum_pool.tile([P, 512], f32, name="ps", tag="psum")
                        for kd in range(KD):
                            nc.tensor.matmul(
                                ps,
                                lhsT=we[:, kd * FL + mf * P: kd * FL + (mf + 1) * P],
                                rhs=xt[:, kd * B + nb * 512: kd * B + nb * 512 + 512],
                                start=(kd == 0),
                                stop=(kd == KD - 1),
                            )
                        nc.scalar.activation(
                            ft[:, nb * 512:(nb + 1) * 512], ps, relu,
                            bias=bt[:, bcol:bcol + 1],
                        )

            # ---------------- decode all destination layers ----------------
            for j in range(L):
                nmm = (j + 1) * MF
                first_pidx = j * (j + 1) // 2
                while emitted < first_pidx + j + 1:
                    emit_wdec(emitted)
                    emitted += 1
                for nb in range(NB):
                    stage = stage_pool.tile([P, MD * 512], bf, name="st", tag="st")
                    for md in range(MD):
                        ps = psum_pool.tile([P, 512], f32, name="ps", tag="psum")
                        c = 0
                        for i in range(j + 1):
                            wt = wd_tiles[first_pidx + i]
                            for kf in range(MF):
                                nc.tensor.matmul(
                                    ps,
                                    lhsT=wt[:, kf * D + md * P: kf * D + (md + 1) * P],
                                    rhs=feats[(i, kf)][:, nb * 512:(nb + 1) * 512],
                                    start=(c == 0),
                                    stop=(c == nmm - 1),
                                )
                                c += 1
                        nc.vector.tensor_copy(stage[:, md * 512:(md + 1) * 512], ps)
                        if md == 2:
                            nc.scalar.dma_start(
                                out_d[j, nb, :, 0:1536], stage[:, 0:1536]
                            )
                    nc.scalar.dma_start(
                        out_d[j, nb, :, 1536:3072], stage[:, 1536:3072]
                    )

    nc.compile()
    return nc


def _prepare_inputs(x, W_enc, b_enc, W_dec):
    """Host-side shard + pack + cast. Returns in_maps for the 8 cores."""
    # x: [L,B,D] -> [L, P, KD*B] with xt[l,p,kd*B+b] = x[l,b,kd*P+p]
    xt = np.ascontiguousarray(
        x.reshape(L, B, KD, P).transpose(0, 3, 2, 1).reshape(L, P, KD * B)
    ).astype(BF16)
    in_maps = []
    for c in range(NCORES):
        s = slice(c * FL, (c + 1) * FL)
        # W_enc[:, s, :]: [L,FL,D] -> [L, P, KD*FL]; we[l,p,kd*FL+f] = W[l,f,kd*P+p]
        wenc = np.ascontiguousarray(
            W_enc[:, s, :].reshape(L, FL, KD, P).transpose(0, 3, 2, 1)
            .reshape(L, P, KD * FL)
        ).astype(BF16)
        # b_enc[:, s]: [L,FL] -> [P, L*MF]; be[p, j*MF+mf] = b[j, mf*P+p]
        benc = np.ascontiguousarray(
            b_enc[:, s].reshape(L, MF, P).transpose(2, 0, 1).reshape(P, L * MF),
            dtype=np.float32,
        )
        # W_dec pairs: [FL,D] -> [P, MF*D]; wd[p, kf*D+d] = W[kf*P+p, d]
        wdec = np.empty((NPAIR, P, MF * D), dtype=BF16)
        for pidx, (i, j) in enumerate(PAIRS):
            wdec[pidx] = (
                W_dec[i, j, s, :].reshape(MF, P, D).transpose(1, 0, 2)
                .reshape(P, MF * D).astype(BF16)
            )
        in_maps.append({"xt": xt, "wenc": wenc, "benc": benc, "wdec": wdec})
    return in_maps


def kernel(x, W_enc, b_enc, W_dec, b_dec):
    global _PROGRAM, LAST_EXEC_NS, LAST_RESULTS
    from concourse import bass_utils

    x = np.asarray(x)
    W_enc = np.asarray(W_enc)
    b_enc = np.asarray(b_enc)
    W_dec = np.asarray(W_dec)
    b_dec = np.asarray(b_dec)

    if _PROGRAM is None:
        _PROGRAM = _build_program()
    nc = _PROGRAM

    in_maps = _prepare_inputs(x, W_enc, b_enc, W_dec)

    trace = os.environ.get("KERNEL_TRACE", "0") == "1"
    res = bass_utils.run_bass_kernel_spmd(
        nc, in_maps, core_ids=list(range(NCORES)), trace=trace
    )
    LAST_EXEC_NS = res.exec_time_ns
    LAST_RESULTS = res

    # out_d: [L, NB, P, MD*512] bf16; full[j, nb*512+b, md*128+p] = acc[j,nb,p,md*512+b]
    acc = np.zeros((L, NB, P, MD * 512), dtype=np.float32)
    for r in res.results:
        acc += np.asarray(r["outT"]).astype(np.float32)
    out = (
        acc.reshape(L, NB, P, MD, 512)
        .transpose(0, 1, 4, 3, 2)
        .reshape(L, B, D)
    )
    out = out + b_dec.astype(np.float32)[:, None, :]
    return np.ascontiguousarray(out, dtype=np.float32)


# revision 7
# speedup vs baseline: 1.0664x; 1.0119x over previous
"""Cross-layer transcoder kernel for Trainium2 (8 NeuronCores, SPMD).

Math (from the reference):
    feats[l] = relu(x[l] @ W_enc[l].T + b_enc[l])           # [B, F] per layer
    recon[j] = sum_{i<=j} feats[i] @ W_dec[i, j] + b_dec[j]  # [B, D] per layer

Sharding: the transcoder feature dim F=4096 is split across the 8 cores
(512 features each). Each core encodes its feature slice for all layers and
computes a partial reconstruction for every destination layer; the partials
are summed on the host (the gather/unshard step), where b_dec is also added.

Schedule (v2): all 8 layer encodes run first, then the 8 triangular decodes.
This front-loads the cheap-to-feed compute (encode needs 2.4MB/layer) and
gives the W_dec stream (28MB/core) an 80us runway to prefetch, so the PE
never waits on HBM after the first ~15us.

DMA layout (v2): every operand is host-packed so one dma_start moves one
whole logical tile with 6-12KB contiguous lines:
  - x:      1 load per layer   [128, 6144] bf16 (12KB lines)
  - W_enc:  1 load per layer   [128, 3072] bf16 (6KB lines)
  - b_enc:  1 load total       [128, 32]   f32
  - W_dec:  1 load per (i,j)   [128, 3072] bf16 (6KB lines), 36 pairs
  - out:    2 stores per (j,nb) of a [128, 3072] bf16 staging tile
This cuts dma_start count 400 -> ~85 (each costs ~650ns of serial issue
time on the Sync/Scalar engine) and roughly doubles effective early HBM
bandwidth vs 1-2KB lines.

Numerics: matmuls in bf16, fp32 PSUM accumulation, bias+relu in fp32 on
the scalar engine writing bf16 feats. Output partials stored as bf16
(error contribution ~0.2% vs the 2e-2 gate; measured total ~3e-3).
"""

import os

import numpy as np
import ml_dtypes

L = 8          # n_layers
B = 1024       # n_pos
D = 768        # d_model
F = 4096       # d_transcoder
NCORES = 8
FL = F // NCORES   # features per core = 512
P = 128
KD = D // P        # 6  encode contraction chunks
MF = FL // P       # 4  feature chunks per core
MD = D // P        # 6  decode output chunks
NB = B // 512      # 2  position chunks of 512
PAIRS = [(i, j) for j in range(L) for i in range(j + 1)]
NPAIR = len(PAIRS)  # 36

WDEC_BUFS = 14      # W_dec SBUF ring slots (6KB/partition each)
N_WARMUP = 26       # PE warmup matmuls bridging the DMA prologue (~15us)

BF16 = ml_dtypes.bfloat16

# Filled by the first kernel() call; reused afterwards.
_PROGRAM = None
# Stash of the most recent run's profiling results (test.py reads these).
LAST_EXEC_NS = None
LAST_RESULTS = None


def _build_program():
    import concourse.bacc as bacc
    import concourse.mybir as mybir
    import concourse.tile as tile

    nc = bacc.Bacc("TRN2", target_bir_lowering=False, debug=False)
    bf = mybir.dt.bfloat16
    f32 = mybir.dt.float32

    xt_d = nc.dram_tensor("xt", [L, P, KD * B], bf, kind="ExternalInput")
    wenc_d = nc.dram_tensor("wenc", [L, P, KD * FL], bf, kind="ExternalInput")
    benc_d = nc.dram_tensor("benc", [P, L * MF], f32, kind="ExternalInput")
    wdec_d = nc.dram_tensor("wdec", [NPAIR, P, MF * D], bf, kind="ExternalInput")
    out_d = nc.dram_tensor("outT", [L, NB, P, MD * 512], bf, kind="ExternalOutput")

    relu = mybir.ActivationFunctionType.Relu

    with tile.TileContext(nc) as tc:
        with (
            tc.tile_pool(name="feats", bufs=1) as feats_pool,
            tc.tile_pool(name="benc", bufs=1) as benc_pool,
            tc.tile_pool(name="xt", bufs=2) as xt_pool,
            tc.tile_pool(name="wenc", bufs=2) as wenc_pool,
            tc.tile_pool(name="wdec", bufs=WDEC_BUFS) as wdec_pool,
            tc.tile_pool(name="stage", bufs=6) as stage_pool,
            tc.tile_pool(name="psum", bufs=8, space="PSUM") as psum_pool,
        ):
            # b_enc: one tiny load, column (j*MF+mf) is the bias vector for
            # that feature chunk.
            bt = benc_pool.tile([P, L * MF], f32, name="benc_t")
            nc.scalar.dma_start(bt, benc_d[:, :])

            # PE warmup during the DMA prologue: first real matmul data
            # lands ~14us in; these dummies keep HAM from holding the PE
            # at 1.2 GHz when the real stream starts.
            warm = benc_pool.tile([P, 512], bf, name="warm")
            nc.vector.memset(warm, 0)
            wps = psum_pool.tile([P, 512], f32, name="wps", tag="psum")
            for w in range(N_WARMUP):
                nc.tensor.matmul(
                    wps,
                    lhsT=warm[:, :P],
                    rhs=warm,
                    start=(w == 0),
                    stop=(w == N_WARMUP - 1),
                )

            feats = {}

            # ---------------- encode all layers ----------------
            for j in range(L):
                xt = xt_pool.tile([P, KD * B], bf, name="xt", tag="xt")
                nc.sync.dma_start(xt, xt_d[j])
                we = wenc_pool.tile([P, KD * FL], bf, name="we", tag="we")
                nc.sync.dma_start(we, wenc_d[j])

                for mf in range(MF):
                    ft = feats_pool.tile([P, B], bf, name=f"feat_{j}_{mf}")
                    feats[(j, mf)] = ft
                    bcol = j * MF + mf
                    for nb in range(NB):
                        ps = psum_pool.tile([P, 512], f32, name="ps", tag="psum")
                        for kd in range(KD):
                            nc.tensor.matmul(
                                ps,
                                lhsT=we[:, kd * FL + mf * P: kd * FL + (mf + 1) * P],
                                rhs=xt[:, kd * B + nb * 512: kd * B + nb * 512 + 512],
                                start=(kd == 0),
                                stop=(kd == KD - 1),
                            )
                        nc.scalar.activation(
                            ft[:, nb * 512:(nb + 1) * 512], ps, relu,
                            bias=bt[:, bcol:bcol + 1],
                        )

            # W_dec loads: all enqueued on the sync queue AFTER the encode
            # loads so they can never delay an x/W_enc transfer (the DMA ring
            # is FIFO). Issues past WDEC_BUFS block the sync engine until
            # decode frees ring slots; nothing else uses sync by then.
            wd_tiles = {}
            for pidx in range(NPAIR):
                wt = wdec_pool.tile([P, MF * D], bf, name="wd", tag="wd")
                nc.sync.dma_start(wt, wdec_d[pidx])
                wd_tiles[pidx] = wt

            # ---------------- decode all destination layers ----------------
            for j in range(L):
                nmm = (j + 1) * MF
                first_pidx = j * (j + 1) // 2
                for nb in range(NB):
                    for md in range(MD):
                        ps = psum_pool.tile([P, 512], f32, name="ps", tag="psum")
                        c = 0
                        for i in range(j + 1):
                            wt = wd_tiles[first_pidx + i]
                            for kf in range(MF):
                                nc.tensor.matmul(
                                    ps,
                                    lhsT=wt[:, kf * D + md * P: kf * D + (md + 1) * P],
                                    rhs=feats[(i, kf)][:, nb * 512:(nb + 1) * 512],
                                    start=(c == 0),
                                    stop=(c == nmm - 1),
                                )
                                c += 1
                        stg = stage_pool.tile([P, 512], bf, name="st", tag="st")
                        nc.vector.tensor_copy(stg, ps)
                        nc.scalar.dma_start(
                            out_d[j, nb, :, md * 512:(md + 1) * 512], stg
                        )

    nc.compile()
    return nc


def _prepare_inputs(x, W_enc, b_enc, W_dec):
    """Host-side shard + pack + cast. Returns in_maps for the 8 cores."""
    # x: [L,B,D] -> [L, P, KD*B] with xt[l,p,kd*B+b] = x[l,b,kd*P+p]
    xt = np.ascontiguousarray(
        x.reshape(L, B, KD, P).transpose(0, 3, 2, 1).reshape(L, P, KD * B)
    ).astype(BF16)
    in_maps = []
    for c in range(NCORES):
        s = slice(c * FL, (c + 1) * FL)
        # W_enc[:, s, :]: [L,FL,D] -> [L, P, KD*FL]; we[l,p,kd*FL+f] = W[l,f,kd*P+p]
        wenc = np.ascontiguousarray(
            W_enc[:, s, :].reshape(L, FL, KD, P).transpose(0, 3, 2, 1)
            .reshape(L, P, KD * FL)
        ).astype(BF16)
        # b_enc[:, s]: [L,FL] -> [P, L*MF]; be[p, j*MF+mf] = b[j, mf*P+p]
        benc = np.ascontiguousarray(
            b_enc[:, s].reshape(L, MF, P).transpose(2, 0, 1).reshape(P, L * MF),
            dtype=np.float32,
        )
        # W_dec pairs: [FL,D] -> [P, MF*D]; wd[p, kf*D+d] = W[kf*P+p, d]
        wdec = np.empty((NPAIR, P, MF * D), dtype=BF16)
        for pidx, (i, j) in enumerate(PAIRS):
            wdec[pidx] = (
                W_dec[i, j, s, :].reshape(MF, P, D).transpose(1, 0, 2)
                .reshape(P, MF * D).astype(BF16)
            )
        in_maps.append({"xt": xt, "wenc": wenc, "benc": benc, "wdec": wdec})
    return in_maps


def kernel(x, W_enc, b_enc, W_dec, b_dec):
    global _PROGRAM, LAST_EXEC_NS, LAST_RESULTS
    from concourse import bass_utils

    x = np.asarray(x)
    W_enc = np.asarray(W_enc)
    b_enc = np.asarray(b_enc)
    W_dec = np.asarray(W_dec)
    b_dec = np.asarray(b_dec)

    if _PROGRAM is None:
        _PROGRAM = _build_program()
    nc = _PROGRAM

    in_maps = _prepare_inputs(x, W_enc, b_enc, W_dec)

    trace = os.environ.get("KERNEL_TRACE", "0") == "1"
    res = bass_utils.run_bass_kernel_spmd(
        nc, in_maps, core_ids=list(range(NCORES)), trace=trace
    )
    LAST_EXEC_NS = res.exec_time_ns
    LAST_RESULTS = res

    # out_d: [L, NB, P, MD*512] bf16; full[j, nb*512+b, md*128+p] = acc[j,nb,p,md*512+b]
    acc = np.zeros((L, NB, P, MD * 512), dtype=np.float32)
    for r in res.results:
        acc += np.asarray(r["outT"]).astype(np.float32)
    out = (
        acc.reshape(L, NB, P, MD, 512)
        .transpose(0, 1, 4, 3, 2)
        .reshape(L, B, D)
    )
    out = out + b_dec.astype(np.float32)[:, None, :]
    return np.ascontiguousarray(out, dtype=np.float32)


# revision 10
# speedup vs baseline: 1.0740x; 1.0071x over previous
"""Cross-layer transcoder kernel for Trainium2 (8 NeuronCores, SPMD).

Math (from the reference):
    feats[l] = relu(x[l] @ W_enc[l].T + b_enc[l])           # [B, F] per layer
    recon[j] = sum_{i<=j} feats[i] @ W_dec[i, j] + b_dec[j]  # [B, D] per layer

Sharding: the transcoder feature dim F=4096 is split across the 8 cores
(512 features each). Each core encodes its feature slice for all layers and
computes a partial reconstruction for every destination layer; the partials
are summed on the host (the gather/unshard step), where b_dec is also added.

Schedule (v2): all 8 layer encodes run first, then the 8 triangular decodes.
This front-loads the cheap-to-feed compute (encode needs 2.4MB/layer) and
gives the W_dec stream (28MB/core) an 80us runway to prefetch, so the PE
never waits on HBM after the first ~15us.

DMA layout (v2): every operand is host-packed so one dma_start moves one
whole logical tile with 6-12KB contiguous lines:
  - x:      1 load per layer   [128, 6144] bf16 (12KB lines)
  - W_enc:  1 load per layer   [128, 3072] bf16 (6KB lines)
  - b_enc:  1 load total       [128, 32]   f32
  - W_dec:  1 load per (i,j)   [128, 3072] bf16 (6KB lines), 36 pairs
  - out:    2 stores per (j,nb) of a [128, 3072] bf16 staging tile
This cuts dma_start count 400 -> ~85 (each costs ~650ns of serial issue
time on the Sync/Scalar engine) and roughly doubles effective early HBM
bandwidth vs 1-2KB lines.

Numerics: matmuls in bf16, fp32 PSUM accumulation, bias+relu in fp32 on
the scalar engine writing bf16 feats. Output partials stored as bf16
(error contribution ~0.2% vs the 2e-2 gate; measured total ~3e-3).
"""

import os

import numpy as np
import ml_dtypes

L = 8          # n_layers
B = 1024       # n_pos
D = 768        # d_model
F = 4096       # d_transcoder
NCORES = 8
FL = F // NCORES   # features per core = 512
P = 128
KD = D // P        # 6  encode contraction chunks
MF = FL // P       # 4  feature chunks per core
MD = D // P        # 6  decode output chunks
NB = B // 512      # 2  position chunks of 512
PAIRS = [(i, j) for j in range(L) for i in range(j + 1)]
NPAIR = len(PAIRS)  # 36

WDEC_BUFS = 14      # W_dec SBUF ring slots (6KB/partition each)
N_WARMUP = 13       # PE warmup matmuls bridging the DMA prologue (~12us)

BF16 = ml_dtypes.bfloat16

# Filled by the first kernel() call; reused afterwards.
_PROGRAM = None
# Stash of the most recent run's profiling results (test.py reads these).
LAST_EXEC_NS = None
LAST_RESULTS = None


def _build_program():
    import concourse.bacc as bacc
    import concourse.mybir as mybir
    import concourse.tile as tile

    nc = bacc.Bacc("TRN2", target_bir_lowering=False, debug=False)
    bf = mybir.dt.bfloat16
    f32 = mybir.dt.float32

    xt_d = nc.dram_tensor("xt", [L, P, KD * B], bf, kind="ExternalInput")
    wenc_d = nc.dram_tensor("wenc", [L, P, KD * FL], bf, kind="ExternalInput")
    benc_d = nc.dram_tensor("benc", [P, L * MF], f32, kind="ExternalInput")
    wdec_d = nc.dram_tensor("wdec", [NPAIR, P, MF * D], bf, kind="ExternalInput")
    out_d = nc.dram_tensor("outT", [L, NB, P, MD * 512], bf, kind="ExternalOutput")

    relu = mybir.ActivationFunctionType.Relu

    with tile.TileContext(nc) as tc:
        with (
            tc.tile_pool(name="feats", bufs=1) as feats_pool,
            tc.tile_pool(name="benc", bufs=1) as benc_pool,
            tc.tile_pool(name="xt", bufs=2) as xt_pool,
            tc.tile_pool(name="wenc", bufs=2) as wenc_pool,
            tc.tile_pool(name="wdec", bufs=WDEC_BUFS) as wdec_pool,
            tc.tile_pool(name="stage", bufs=6) as stage_pool,
            tc.tile_pool(name="psum", bufs=8, space="PSUM") as psum_pool,
        ):
            # b_enc: one tiny load, column (j*MF+mf) is the bias vector for
            # that feature chunk.
            bt = benc_pool.tile([P, L * MF], f32, name="benc_t")
            nc.scalar.dma_start(bt, benc_d[:, :])

            # PE warmup during the DMA prologue: first real matmul data
            # lands ~14us in; these dummies keep HAM from holding the PE
            # at 1.2 GHz when the real stream starts.
            warm = benc_pool.tile([P, 512], bf, name="warm")
            nc.vector.memset(warm, 0)
            wps = psum_pool.tile([P, 512], f32, name="wps", tag="psum")
            for w in range(N_WARMUP):
                nc.tensor.matmul(
                    wps,
                    lhsT=warm[:, :P],
                    rhs=warm,
                    start=(w == 0),
                    stop=(w == N_WARMUP - 1),
                )

            feats = {}

            # ---------------- encode all layers ----------------
            # Layer 0 is latency-critical: its loads are split into kd-halves
            # and its 8 psum chains into two accumulation passes, so the PE
            # can start on the first half (~1.1MB) while the second half is
            # still in flight. (Subtile dependency tracking scopes each
            # matmul's wait to the DMA that wrote its kd range.)
            for j in range(L):
                xt = xt_pool.tile([P, KD * B], bf, name="xt", tag="xt")
                we = wenc_pool.tile([P, KD * FL], bf, name="we", tag="we")
                if j == 0:
                    h = KD // 2
                    nc.sync.dma_start(xt[:, :h * B], xt_d[j][:, :h * B])
                    nc.sync.dma_start(we[:, :h * FL], wenc_d[j][:, :h * FL])
                    nc.sync.dma_start(xt[:, h * B:], xt_d[j][:, h * B:])
                    nc.sync.dma_start(we[:, h * FL:], wenc_d[j][:, h * FL:])
                    kd_passes = [range(h), range(h, KD)]
                else:
                    nc.sync.dma_start(xt, xt_d[j])
                    nc.sync.dma_start(we, wenc_d[j])
                    kd_passes = [range(KD)]

                chains = {}
                for kds in kd_passes:
                    for mf in range(MF):
                        if (j, mf) not in feats:
                            ft = feats_pool.tile([P, B], bf, name=f"feat_{j}_{mf}")
                            feats[(j, mf)] = ft
                        for nb in range(NB):
                            if (mf, nb) not in chains:
                                chains[(mf, nb)] = psum_pool.tile(
                                    [P, 512], f32, name="ps", tag="psum"
                                )
                            ps = chains[(mf, nb)]
                            for kd in kds:
                                nc.tensor.matmul(
                                    ps,
                                    lhsT=we[:, kd * FL + mf * P: kd * FL + (mf + 1) * P],
                                    rhs=xt[:, kd * B + nb * 512: kd * B + nb * 512 + 512],
                                    start=(kd == 0),
                                    stop=(kd == KD - 1),
                                )
                            if kd == KD - 1:
                                nc.scalar.activation(
                                    feats[(j, mf)][:, nb * 512:(nb + 1) * 512],
                                    ps, relu,
                                    bias=bt[:, j * MF + mf: j * MF + mf + 1],
                                )

            # W_dec loads: all enqueued on the sync queue AFTER the encode
            # loads so they can never delay an x/W_enc transfer (the DMA ring
            # is FIFO). Issues past WDEC_BUFS block the sync engine until
            # decode frees ring slots; nothing else uses sync by then.
            wd_tiles = {}
            for pidx in range(NPAIR):
                wt = wdec_pool.tile([P, MF * D], bf, name="wd", tag="wd")
                nc.sync.dma_start(wt, wdec_d[pidx])
                wd_tiles[pidx] = wt

            # ---------------- decode all destination layers ----------------
            for j in range(L):
                nmm = (j + 1) * MF
                first_pidx = j * (j + 1) // 2
                for nb in range(NB):
                    for md in range(MD):
                        ps = psum_pool.tile([P, 512], f32, name="ps", tag="psum")
                        c = 0
                        for i in range(j + 1):
                            wt = wd_tiles[first_pidx + i]
                            for kf in range(MF):
                                nc.tensor.matmul(
                                    ps,
                                    lhsT=wt[:, kf * D + md * P: kf * D + (md + 1) * P],
                                    rhs=feats[(i, kf)][:, nb * 512:(nb + 1) * 512],
                                    start=(c == 0),
                                    stop=(c == nmm - 1),
                                )
                                c += 1
                        stg = stage_pool.tile([P, 512], bf, name="st", tag="st")
                        nc.vector.tensor_copy(stg, ps)
                        # Final layer's stores ride the (idle by then) sync
                        # queue so the last store isn't stuck behind the
                        # scalar store ring's backlog.
                        eng = nc.sync if j == L - 1 else nc.scalar
                        eng.dma_start(
                            out_d[j, nb, :, md * 512:(md + 1) * 512], stg
                        )

    nc.compile()
    return nc


def _prepare_inputs(x, W_enc, b_enc, W_dec):
    """Host-side shard + pack + cast. Returns in_maps for the 8 cores."""
    # x: [L,B,D] -> [L, P, KD*B] with xt[l,p,kd*B+b] = x[l,b,kd*P+p]
    xt = np.ascontiguousarray(
        x.reshape(L, B, KD, P).transpose(0, 3, 2, 1).reshape(L, P, KD * B)
    ).astype(BF16)
    in_maps = []
    for c in range(NCORES):
        s = slice(c * FL, (c + 1) * FL)
        # W_enc[:, s, :]: [L,FL,D] -> [L, P, KD*FL]; we[l,p,kd*FL+f] = W[l,f,kd*P+p]
        wenc = np.ascontiguousarray(
            W_enc[:, s, :].reshape(L, FL, KD, P).transpose(0, 3, 2, 1)
            .reshape(L, P, KD * FL)
        ).astype(BF16)
        # b_enc[:, s]: [L,FL] -> [P, L*MF]; be[p, j*MF+mf] = b[j, mf*P+p]
        benc = np.ascontiguousarray(
            b_enc[:, s].reshape(L, MF, P).transpose(2, 0, 1).reshape(P, L * MF),
            dtype=np.float32,
        )
        # W_dec pairs: [FL,D] -> [P, MF*D]; wd[p, kf*D+d] = W[kf*P+p, d]
        wdec = np.empty((NPAIR, P, MF * D), dtype=BF16)
        for pidx, (i, j) in enumerate(PAIRS):
            wdec[pidx] = (
                W_dec[i, j, s, :].reshape(MF, P, D).transpose(1, 0, 2)
                .reshape(P, MF * D).astype(BF16)
            )
        in_maps.append({"xt": xt, "wenc": wenc, "benc": benc, "wdec": wdec})
    return in_maps


def kernel(x, W_enc, b_enc, W_dec, b_dec):
    global _PROGRAM, LAST_EXEC_NS, LAST_RESULTS
    from concourse import bass_utils

    x = np.asarray(x)
    W_enc = np.asarray(W_enc)
    b_enc = np.asarray(b_enc)
    W_dec = np.asarray(W_dec)
    b_dec = np.asarray(b_dec)

    if _PROGRAM is None:
        _PROGRAM = _build_program()
    nc = _PROGRAM

    in_maps = _prepare_inputs(x, W_enc, b_enc, W_dec)

    trace = os.environ.get("KERNEL_TRACE", "0") == "1"
    res = bass_utils.run_bass_kernel_spmd(
        nc, in_maps, core_ids=list(range(NCORES)), trace=trace
    )
    LAST_EXEC_NS = res.exec_time_ns
    LAST_RESULTS = res

    # out_d: [L, NB, P, MD*512] bf16; full[j, nb*512+b, md*128+p] = acc[j,nb,p,md*512+b]
    acc = np.zeros((L, NB, P, MD * 512), dtype=np.float32)
    for r in res.results:
        acc += np.asarray(r["outT"]).astype(np.float32)
    out = (
        acc.reshape(L, NB, P, MD, 512)
        .transpose(0, 1, 4, 3, 2)
        .reshape(L, B, D)
    )
    out = out + b_dec.astype(np.float32)[:, None, :]
    return np.ascontiguousarray(out, dtype=np.float32)
